# revision 8
# baseline (speedup 1.0000x reference)
"""Trainium2 Bass kernel for MHSA with relative position bias (nn_MHSARPB).

Problem (hardcoded): x (8, 32, 32, 512), qkv_w (1536, 512), qkv_b (1536,),
rpb (16, 63, 63), proj_w (512, 512), proj_b (512,). Output (8, 32, 32, 512) f32.

Strategy: tensor-parallel over the 16 heads -> 2 heads per core on 8 cores.
Each core computes q/k/v for its 2 heads over all 8*1024 tokens, the full
attention for its (8 batches x 2 heads) pairs, and a partial projection
output (contraction over its 64 channels). The host sums the 8 partial
projection outputs and adds proj_b (+ proj_w @ bv, see below).

Key engineering vs the straightforward version:
  - S^T = k^T q runs in fp8e4 with perf_mode=DoubleRow (2 fp8 weights per PE
    cell): q,k are drained from the qkv PSUM straight to fp8 in a
    [p, 2, T] two-k-tile layout whose second tile is all zeros (DoubleRow
    contracts over both tiles; zeros make it mathematically a plain K=32
    matmul at 0.5 cycles/column). ~1.4x tensor-engine win on the dominant
    matmul. q (parts 0-63) is used in place; k (parts 64-127) is relocated
    once per 2-batch group to partitions 0-63 by a small SBUF->SBUF DMA so
    stationary/moving share partitions, as the PE requires.
  - v-bias is folded into proj_b on the host (softmax weights sum to 1, so
    the +bv shows up as +proj_w@bv on the output); the v drain is a plain
    copy on the Activation engine.
  - softmax exp: exp(S)*exp(bias) with exp(bias) precomputed on host (fp16,
    resident). exp(S) runs on the Activation engine for most (pair, m-chunk)
    blocks; every SCHR_MOD-th block instead uses a Schraudolph bit-trick on
    DVE (i16 = S*1477.32 + 15360.5, bit-cast to fp16 == 2^~(S*log2e)) to
    offload the Activation engine. The exp(bias) multiply runs on DVE
    (fp16 4x mode), with every GP_MOD-th on GpSimd.
  - AV keeps V (64-col blocks incl. a ones-column for the softmax
    denominator) stationary and streams E; 2 heads are column-packed.
  - norm: denominators go through a DRAM gather -> reciprocal -> stride-0
    partition broadcast, then one y *= r pass split across DVE and GpSimd.
  - proj: partial out^T (co, t) with zero weight rows annihilating the
    denominator/junk rows; PSUM drained half by DVE, half by ScalarE.
"""
import sys

sys.path.insert(0, "/opt/trn_rl_repo")

import contextlib
import numpy as np
import concourse.bass as bass
import concourse.bacc as bacc
import concourse.tile as tile
from concourse import mybir
from concourse.bass_utils import run_bass_kernel_spmd
from concourse.tile_rust import add_dep_helper

FP16 = mybir.dt.float16
FP32 = mybir.dt.float32
FP8 = mybir.dt.float8e4
I16 = mybir.dt.int16
EXP = mybir.ActivationFunctionType.Exp
DR = mybir.MatmulPerfMode.DoubleRow
MUL = mybir.AluOpType.mult
ADD = mybir.AluOpType.add

B, S, C, NH = 8, 32, 512, 16
N = S * S            # 1024 tokens per image
T = B * N            # 8192 tokens
D = C // NH          # 32 head dim
SCALE = D ** -0.5
N_CORES = 8

# Schraudolph exp constants: i16 = S*1024*log2(e) + 15360.5, bitcast fp16.
SCHR_C1 = float(1024.0 * np.log2(np.e))
SCHR_C2 = 15360.5
SCHR_MOD = 4         # every SCHR_MOD-th (pair, m-chunk) uses DVE schraudolph
GP_MOD = 8           # every GP_MOD-th exp(bias) multiply runs on GpSimd

_CACHE = {}


def build_nc(repeat=1, debug_out=False):
    nc = bacc.Bacc("TRN2", target_bir_lowering=False, debug=False)

    xT = nc.dram_tensor("xT", [C, T], FP16, kind="ExternalInput")
    wqkT = nc.dram_tensor("wqkT", [4, 128, 128], FP16, kind="ExternalInput")
    wvT = nc.dram_tensor("wvT", [4, 128, 64], FP16, kind="ExternalInput")
    bqk = nc.dram_tensor("bqk", [128, 1], FP32, kind="ExternalInput")
    expb = nc.dram_tensor("expb", [128, 16384], FP16, kind="ExternalInput")
    projT = nc.dram_tensor("projT", [128, 512], FP16, kind="ExternalInput")
    outT = nc.dram_tensor("outT", [C, T], FP16, kind="ExternalOutput")
    den_scr = nc.dram_tensor("den_scr", [2, 8192], FP16)
    denr_scr = nc.dram_tensor("denr_scr", [2, 8192], FP16)
    if debug_out:
        dbg_qk8 = nc.dram_tensor("dbg_qk8", [128, 2 * T], FP8,
                                 kind="ExternalOutput")
        dbg_k8 = nc.dram_tensor("dbg_k8", [128, 2 * T], FP8,
                                kind="ExternalOutput")
        dbg_v = nc.dram_tensor("dbg_v", [128, T], FP16, kind="ExternalOutput")
        dbg_y = nc.dram_tensor("dbg_y", [128, T], FP16, kind="ExternalOutput")

    with tile.TileContext(nc) as tc:
        with (
            tc.tile_pool(name="consts", bufs=1) as consts,
            tc.tile_pool(name="big", bufs=1) as big,
            tc.tile_pool(name="xin", bufs=4) as xin,
            tc.tile_pool(name="epool", bufs=6) as epool,
            tc.tile_pool(name="stpool", bufs=4) as stpool,
        ):
            # ---- constants (loaded ONCE, outside the repeat loop) ----------
            wqk_sb = consts.tile([128, 4 * 128], FP16, tag="wqk_sb")
            nc.sync.dma_start(
                out=wqk_sb.rearrange("p (kc f) -> p kc f", kc=4),
                in_=wqkT[:].transpose([1, 0, 2]),
            )
            wv_sb = consts.tile([128, 4 * 64], FP16, tag="wv_sb")
            nc.sync.dma_start(
                out=wv_sb.rearrange("p (kc f) -> p kc f", kc=4),
                in_=wvT[:].transpose([1, 0, 2]),
            )
            bqk_sb = consts.tile([128, 1], FP32, tag="bqk_sb")
            nc.sync.dma_start(out=bqk_sb[:], in_=bqk[:])
            expb_sb = consts.tile([128, 16384], FP16, tag="expb_sb")
            nc.scalar.dma_start(out=expb_sb[:, 0:8192], in_=expb[:, 0:8192])
            nc.scalar.dma_start(out=expb_sb[:, 8192:16384],
                                in_=expb[:, 8192:16384])
            projT_sb = consts.tile([128, 512], FP16, tag="projT_sb")
            nc.scalar.dma_start(out=projT_sb[:], in_=projT[:])

            # ---- persistent big tensors -----------------------------------
            # qk8: parts 0-31 q_h0, 32-63 q_h1 (rhs in place);
            #      parts 64-95 k_h0, 96-127 k_h1 (staging, DMA'd to k8).
            # layout [p, ktile 2, T]; ktile 1 is all zeros (DoubleRow pad).
            qk8 = big.tile([128, 2, T], FP8, tag="qk8")
            k8 = big.tile([128, 2, T], FP8, tag="k8")
            v_nat = big.tile([128, T], FP16, tag="v_nat")   # (hi, b, j) 64-col blocks
            y_sb = big.tile([128, T], FP16, tag="y_sb")
            r_bcast = big.tile([128, T], FP16, tag="r_bcast")

            nc.gpsimd.memset(qk8[:, 1, :], 0.0)
            nc.gpsimd.memset(k8[:, 1, :], 0.0)

            v_blocks = v_nat.rearrange("p (blk cc) -> p blk cc", cc=64)
            nc.gpsimd.memset(v_blocks[:, :, 32:64], 0.0)
            nc.gpsimd.memset(v_blocks[:, :, 32:33], 1.0)
            # drain view for batch b: dims (j, hi, col)
            v_drain = v_nat.rearrange("p (hi b j col) -> p b j hi col",
                                      hi=2, b=8, j=8)

            rep_ctx = (tc.For_i(0, repeat, 1) if repeat > 1
                       else contextlib.nullcontext())
            with rep_ctx:
              with (
                tc.tile_pool(name="ps_s", bufs=2, space="PSUM") as ps_s,
                tc.tile_pool(name="ps_qk", bufs=1, space="PSUM") as ps_qk,
                tc.tile_pool(name="ps_v", bufs=1, space="PSUM") as ps_v,
                tc.tile_pool(name="ps_av", bufs=2, space="PSUM") as ps_av,
              ):
                eidx = [0]
                # FWL<->DoubleRow serialization fences: rapid alternation of
                # FWL weight loads (128-col fp16 stationaries: qk/v/proj)
                # with fp8 DoubleRow matmuls in the PE pipeline is an
                # NRT_EXEC_UNIT_UNRECOVERABLE hardware fault. These edges
                # make the first matmul of each mode-block wait on a drain
                # instruction that post-dates the other block's completion.
                qkv_fence = [None]   # v-drain (Act) of the latest qkv block
                att_fence = [None]   # y-drain (DVE) of the latest att block

                def qkv_batch(b):
                    xts = []
                    for cc in range(2):
                        c = 2 * b + cc
                        xt = xin.tile([128, 2048], FP16, tag="xt")
                        nc.sync.dma_start(
                            out=xt.rearrange("p (kc f) -> p kc f", kc=4),
                            in_=xT.rearrange("(kc p) t -> p kc t", p=128)[
                                :, :, c * 512 : (c + 1) * 512
                            ],
                        )
                        xts.append(xt)
                        psqk = ps_qk.tile([128, 512], FP32, tag="psqk")
                        for kc in range(4):
                            mm = nc.tensor.matmul(
                                psqk[:],
                                wqk_sb[:, kc * 128 : (kc + 1) * 128],
                                xt[:, kc * 512 : (kc + 1) * 512],
                                start=(kc == 0), stop=(kc == 3),
                            )
                            if cc == 0 and kc == 0 and att_fence[0] is not None:
                                add_dep_helper(mm.ins, att_fence[0].ins,
                                               sync=True,
                                               reason="dr->fwl fence")
                        nc.vector.tensor_scalar_add(
                            qk8[:, 0, c * 512 : (c + 1) * 512], psqk[:],
                            bqk_sb[:],
                        )
                    # token-major v (see baseline docstring); bias folded out.
                    psv = ps_v.tile([128, 512], FP32, tag="psv")
                    for cc in range(2):
                        for mc in range(4):
                            ci = cc * 4 + mc
                            for kc in range(4):
                                nc.tensor.matmul(
                                    psv[:, ci * 64 : ci * 64 + 64],
                                    xts[cc][:, kc * 512 + mc * 128 :
                                            kc * 512 + mc * 128 + 128],
                                    wv_sb[:, kc * 64 : (kc + 1) * 64],
                                    start=(ci == 0 and kc == 0),
                                    stop=(ci == 7 and kc == 3),
                                    skip_group_check=True,
                                )
                    qkv_fence[0] = nc.scalar.copy(
                        v_drain[:, b, :, :, 0:32],
                        psv.rearrange("p (j hi d) -> p j hi d", j=8, hi=2),
                    )

                def kdma(b0):
                    # k_h* staging (parts 64-127) -> k8 parts 0-63, 2 batches
                    nc.sync.dma_start(
                        out=k8[0:64, 0, b0 * 1024 : (b0 + 2) * 1024],
                        in_=qk8[64:128, 0, b0 * 1024 : (b0 + 2) * 1024],
                    )

                def attention_batch(b):
                    av0 = ps_av.tile([128, 512], FP32, tag="av")
                    av1 = ps_av.tile([128, 512], FP32, tag="av")
                    avs = [av0, av1]
                    for mc in range(8):
                        for hi in range(2):
                            sps = ps_s.tile([128, 1024], FP32, tag="sps")
                            m0 = b * 1024 + mc * 128
                            for half in range(2):
                                mm = nc.tensor.matmul(
                                    sps[:, half * 512 : (half + 1) * 512],
                                    k8[32 * hi : 32 * hi + 32, :,
                                       m0 : m0 + 128],
                                    qk8[32 * hi : 32 * hi + 32, :,
                                        b * 1024 + half * 512 :
                                        b * 1024 + half * 512 + 512],
                                    start=True, stop=True,
                                    perf_mode=DR,
                                    tile_position=(32 * hi, 0),
                                )
                                if (mc == 0 and hi == 0 and half == 0
                                        and qkv_fence[0] is not None):
                                    add_dep_helper(mm.ins, qkv_fence[0].ins,
                                                   sync=True,
                                                   reason="fwl->dr fence")
                            e_t = epool.tile([128, 1024], FP16, tag="E")
                            i = eidx[0]
                            eidx[0] += 1
                            if i % SCHR_MOD == 1:
                                nc.vector.tensor_scalar(
                                    e_t[:].bitcast(I16), sps[:],
                                    SCHR_C1, SCHR_C2, MUL, ADD,
                                )
                            else:
                                nc.scalar.activation(e_t[:], sps[:], EXP)
                            blk = hi * 8 + mc
                            meng = (nc.gpsimd if i % GP_MOD == 3
                                    else nc.vector)
                            meng.tensor_mul(
                                e_t[:], e_t[:],
                                expb_sb[:, blk * 1024 : (blk + 1) * 1024],
                            )
                            vblk = (hi * 8 + b) * 8 + mc
                            for half in range(2):
                                nc.tensor.matmul(
                                    avs[half][64 * hi : 64 * hi + 64, :],
                                    v_nat[:, vblk * 64 : vblk * 64 + 64],
                                    e_t[:, half * 512 : (half + 1) * 512],
                                    start=(mc == 0), stop=(mc == 7),
                                    tile_position=(0, 64 * hi),
                                    skip_group_check=True,
                                )
                    for half in range(2):
                        att_fence[0] = nc.vector.tensor_copy(
                            y_sb[:, b * 1024 + half * 512 :
                                 b * 1024 + half * 512 + 512],
                            avs[half][:],
                        )

                qkv_batch(0)
                qkv_batch(1)
                kdma(0)
                attention_batch(0)
                qkv_batch(2)
                qkv_batch(3)
                kdma(2)
                attention_batch(1)
                attention_batch(2)
                qkv_batch(4)
                qkv_batch(5)
                kdma(4)
                attention_batch(3)
                attention_batch(4)
                qkv_batch(6)
                qkv_batch(7)
                kdma(6)
                attention_batch(5)
                attention_batch(6)
                attention_batch(7)
                if debug_out:
                    nc.sync.dma_start(
                        out=dbg_qk8[:],
                        in_=qk8.rearrange("p t c -> p (t c)"))
                    nc.sync.dma_start(
                        out=dbg_k8[:],
                        in_=k8.rearrange("p t c -> p (t c)"))
                    nc.sync.dma_start(out=dbg_v[:], in_=v_nat[:])
                    nc.sync.dma_start(out=dbg_y[:], in_=y_sb[:])

              # ---- normalize: den rows -> DRAM -> gather to partitions ->
              # reciprocal -> scatter -> stride-0 broadcast.
              for hi in range(2):
                  nc.sync.dma_start(
                      out=den_scr[hi : hi + 1, :],
                      in_=y_sb[32 + 64 * hi : 33 + 64 * hi, :],
                  )
              den_g = stpool.tile([128, 128], FP32, tag="den_g")
              nc.gpsimd.dma_start(
                  out=den_g[:],
                  in_=den_scr[:].rearrange("h (z n) -> (h z) n", n=128),
              )
              den_r = stpool.tile([128, 128], FP32, tag="den_r")
              nc.vector.reciprocal(den_r[:], den_g[:])
              nc.gpsimd.dma_start(
                  out=denr_scr[:].rearrange("h (z n) -> (h z) n", n=128),
                  in_=den_r[:],
              )
              for hi in range(2):
                  nc.sync.dma_start(
                      out=r_bcast[64 * hi : 64 * hi + 64, :],
                      in_=bass.AP(
                          tensor=denr_scr,
                          offset=hi * 8192,
                          ap=[[0, 64], [1, 8192]],
                      ),
                  )
              nc.vector.tensor_mul(y_sb[:, 0:5120], y_sb[:, 0:5120],
                                   r_bcast[:, 0:5120])
              nc.gpsimd.tensor_mul(y_sb[:, 5120:8192], y_sb[:, 5120:8192],
                                   r_bcast[:, 5120:8192])

              # ---- projection ------------------------------------------------
              with (
                  tc.tile_pool(name="ps_pj", bufs=4, space="PSUM") as ps_pj,
                  tc.tile_pool(name="opool", bufs=4) as opool,
              ):
                  for cs in range(4):
                      for cp in range(8):
                          pj = ps_pj.tile([128, 1024], FP32, tag="pj")
                          for h in range(2):
                              nc.tensor.matmul(
                                  pj[:, h * 512 : (h + 1) * 512],
                                  projT_sb[:, cs * 128 : (cs + 1) * 128],
                                  y_sb[:, (2 * cp + h) * 512 :
                                       (2 * cp + h + 1) * 512],
                                  start=True, stop=True,
                              )
                          o_t = opool.tile([128, 1024], FP16, tag="o_t")
                          i = cs * 8 + cp
                          if i % 2 == 0:
                              nc.vector.tensor_copy(o_t[:], pj[:])
                          else:
                              nc.scalar.copy(o_t[:], pj[:])
                          nc.scalar.dma_start(
                              out=outT[cs * 128 : (cs + 1) * 128,
                                       cp * 1024 : (cp + 1) * 1024],
                              in_=o_t[:],
                          )
    nc.compile()
    return nc


def _prep_inputs(x, qkv_w, qkv_b, rpb, proj_w, proj_b):
    x = np.asarray(x, np.float32)
    qkv_w = np.asarray(qkv_w, np.float32)
    qkv_b = np.asarray(qkv_b, np.float32)
    rpb = np.asarray(rpb, np.float32)

    xT16 = np.ascontiguousarray(x.reshape(T, C).T).astype(np.float16)
    mi = (np.arange(N) // S)[:, None]
    mj = (np.arange(N) % S)[:, None]
    ni = (np.arange(N) // S)[None, :]
    nj = (np.arange(N) % S)[None, :]

    in_maps = []
    for core in range(N_CORES):
        h0, h1 = 2 * core, 2 * core + 1
        rq = list(range(h0 * D, h0 * D + D)) + list(range(h1 * D, h1 * D + D))
        wq = qkv_w[rq, :] * SCALE
        wk = qkv_w[[C + r for r in rq], :]
        wv = qkv_w[[2 * C + r for r in rq], :]
        bq = qkv_b[rq] * SCALE
        bk = qkv_b[[C + r for r in rq]]

        wqk = np.concatenate([wq, wk], axis=0)           # (128, 512)
        wqkT16 = np.ascontiguousarray(wqk.T).astype(np.float16).reshape(4, 128, 128)
        wvT16 = np.ascontiguousarray(wv.T).astype(np.float16).reshape(4, 128, 64)
        bqk_in = np.concatenate([bq, bk]).astype(np.float32).reshape(128, 1)

        # expb block layout: (hi, mc) blocks of (128 m-part, 1024 n)
        expb_in = np.zeros((128, 16384), np.float16)
        for hi, h in enumerate((h0, h1)):
            biasT = rpb[h][31 + mi - ni, 31 + mj - nj]    # (m, n) = bias^T
            eb = np.exp(biasT).astype(np.float16)         # (1024, 1024)
            for mc in range(8):
                blk = hi * 8 + mc
                expb_in[:, blk * 1024 : (blk + 1) * 1024] = (
                    eb[mc * 128 : (mc + 1) * 128, :]
                )

        projT_in = np.zeros((128, 512), np.float16)
        projT_in[0:32] = proj_w[:, 64 * core : 64 * core + 32].T.astype(np.float16)
        projT_in[64:96] = proj_w[:, 64 * core + 32 : 64 * core + 64].T.astype(np.float16)

        in_maps.append({
            "xT": xT16,
            "wqkT": wqkT16,
            "wvT": wvT16,
            "bqk": bqk_in,
            "expb": expb_in,
            "projT": projT_in,
        })
    return in_maps


def kernel(x, qkv_w, qkv_b, rpb, proj_w, proj_b):
    if "nc" not in _CACHE:
        _CACHE["nc"] = build_nc()
    nc = _CACHE["nc"]
    in_maps = _prep_inputs(x, qkv_w, qkv_b, rpb, proj_w, proj_b)
    res = run_bass_kernel_spmd(nc, in_maps, list(range(N_CORES)))
    out = np.zeros((T, C), np.float32)
    for core in range(N_CORES):
        out += res.results[core]["outT"].astype(np.float32).T
    proj_w = np.asarray(proj_w, np.float32)
    bv = np.asarray(qkv_b, np.float32)[2 * C : 3 * C]
    out += (np.asarray(proj_b, np.float32) + proj_w @ bv)[None, :]
    return out.reshape(B, S, S, C)


if __name__ == "__main__":
    rng = np.random.default_rng(0)
    ins = {
        "x": rng.standard_normal((B, S, S, C)).astype(np.float32),
        "qkv_w": (rng.standard_normal((3 * C, C)) * 0.02).astype(np.float32),
        "qkv_b": (rng.standard_normal((3 * C,)) * 0.02).astype(np.float32),
        "rpb": (rng.standard_normal((NH, 2 * S - 1, 2 * S - 1)) * 0.02).astype(np.float32),
        "proj_w": (rng.standard_normal((C, C)) * 0.02).astype(np.float32),
        "proj_b": (rng.standard_normal((C,)) * 0.02).astype(np.float32),
    }
    out = kernel(**ins)
    print("kernel ran, out", out.shape, out.dtype, float(np.abs(out).max()))
